# revision 18
# baseline (speedup 1.0000x reference)
"""Trainium2 Bass kernel for ConvexLinearAttention (elu(x)+1 linear attention).

Full-input contract: kernel(**inputs) takes the unsharded tensors
(x [2,2048,1024], wq/wk/wv/wo [1024,1024], bq/bk/bv/bo [1024]) and returns the
full output [2,2048,1024].

Sharding (8 cores): data-parallel over batch (2) x head-group-parallel (4 groups
of 4 heads).  Each core projects only its 256-wide head slice, runs the
linearized attention (attended = qf @ (kf^T V) / (qf @ sum(kf)) -- an exact
refactoring of the dense normalized scores), and emits a partial output
projection in natural [s, e] layout.  The host sums the 4 head-group partials
per batch.

Precision plan (tolerance 2e-2, bf16 baseline err ~4e-3):
  - K|V projection stays bf16: V weight-quantization error is common-mode
    across sequence positions (x_bar @ dW) and does NOT average out in the
    attention mean -- fp8 V alone measured 3.3e-2.
  - Q projection: fp8e4 DoubleRow (wq8 stationary [128,2,128], x8 moving
    [128,2,512]).  Normalization cancels most q-side noise: 7.8e-3 alone.
  - Out projection: fp8e4 DoubleRow, one MM per (s-tile, e-half) pairing the
    two 128-wide head groups in the ko dim.  qs and M have disjoint dynamic
    ranges (qs ~ 7e-6, M ~ 1e2), so split scales: qs*2^17 (folded into the
    scaled ksum feeding the denominator matmul) and M*2^-6 (folded into the
    M PSUM evacuation); the out PSUM evacuation multiplies by 2^-11.
    Q+out fp8 together measured 1.7e-2 in simulation vs the 2e-2 gate.

DMA plan (per-queue throughput is descriptor-paced: ~35ns + ~60ns/KB per
descriptor, so both the dram AND sbuf side of every transfer must be
contiguous in 2KB+ runs):
  - xb (bf16 x for the K|V stationary) is tile-major on BOTH sides:
    dram rows (st p) hold [do s] 2KB runs, sbuf is [P, NST, ndt, P].
  - x8 (fp8 x, Q moving operand) is chunk-major: dram rows (sc p) hold
    [do s] 4KB runs, sbuf is [P, NSC, ndt, SC].
  - wkv streams as 2 halves on different queues so the dt-ordered matmul
    consumption starts after ~512KB.
  - Input triggers ride only sync/gpsimd: a dma_start occupies a slot in its
    engine's instruction stream and can block behind ring semaphores, so the
    scalar/vector queues (which carry the latency-critical feature ops) get
    no input DMAs after the first four.
  - 12 warm-up matmuls on resident data run during the input head so the PE
    HAM clock gate is already 8/8 when real work starts.
"""

from contextlib import ExitStack

import numpy as np
import ml_dtypes

import concourse.bass as bass
import concourse.mybir as mybir
import concourse.tile as tile
from concourse import bacc, bass_utils

F32 = mybir.dt.float32
BF16 = mybir.dt.bfloat16
FP8 = mybir.dt.float8e4
AF = mybir.ActivationFunctionType
ALU = mybir.AluOpType
DR = mybir.MatmulPerfMode.DoubleRow

NPBF = ml_dtypes.bfloat16
NPF8 = ml_dtypes.float8_e4m3  # IEEE-style e4m3 (max 240) = TRN float8e4

D = 1024          # model dim
S = 2048          # sequence length
BATCH = 2
CSL = 256         # head-slice width per core (4 heads x 64)
NG = 2            # 128-wide c-groups per core
P = 128
NST = S // P      # 16 s-tiles
SC = 512          # s-chunk
NSC = S // SC     # 4 s-chunks
STC = SC // P     # 4 s-tiles per chunk
EH = 512          # e-half width for out-proj rhs

Q_FP8 = True      # fp8 DoubleRow Q projection
# fp8 DoubleRow out projection (split scales).  Measured SLOWER than bf16:
# the single-MM-per-bank DoubleRow stream pays a serialized 256-col
# LDWEIGHTS per matmul (376ns vs 216) plus inter-MM gaps that let the HAM
# clock gate oscillate; the bf16 2-MM accumulation groups stream at 216ns
# with FWL-hidden weight loads.  Kept as a switch for reference.
OUT_FP8 = False
QS_SCALE = 2.0 ** 17   # qs -> qs * QS_SCALE (folded into scaled ksum)
M_SCALE = 2.0 ** -6    # M  -> M * M_SCALE  (folded into M PSUM evac)
O_SCALE = 1.0 / (QS_SCALE * M_SCALE)  # out evac multiplier (2^-11)
NWARM = 16

_CACHE: dict = {}


def install_ntff_hook_shim():
    """Provide ``antenv.axon_hooks`` when the image ships only the antenv stub.

    concourse.bass_utils imports it unconditionally on the axon trace path;
    without this shim trace=True (or BASS_TRACE=1) crashes.  Registers the real
    ctypes NTFF hook when the axon .so is present, else a None-returning stub
    so tracing degrades gracefully.
    """
    import os
    import sys
    import types

    if "antenv.axon_hooks" in sys.modules:
        return
    try:
        import antenv
        import antenv.axon_hooks  # noqa: F401
        return  # real module exists
    except ImportError:
        pass
    mod = types.ModuleType("antenv.axon_hooks")
    state: dict = {"h": None}
    mod.set_axon_ntff_profile_hook = lambda h: state.__setitem__("h", h)
    mod.get_axon_ntff_profile_hook = lambda: state.get("h")
    sys.modules["antenv.axon_hooks"] = mod
    antenv.axon_hooks = mod
    so_path = "/opt/axon/libaxon_pjrt.so"
    if os.path.exists(so_path):
        try:
            from trn_agent_boot.trn_boot import _ntff_profile_via_ctypes

            state["h"] = _ntff_profile_via_ctypes(so_path)
        except Exception:
            pass


def _build_kernel_body(ctx: ExitStack, tc: tile.TileContext, t, use_biases):
    nc = tc.nc
    # with biases, a 9th d-tile (ones row 0, zeros elsewhere) multiplies the
    # bias row appended to the weight matrices: exact bias add inside the GEMM
    ndt = 9 if use_biases else 8

    xbv = t["xb"].ap().rearrange("(st p) (do s) -> p st do s", p=P, do=8)
    x8v = t["x8"].ap().rearrange("(sc p) (do s) -> p sc do s", p=P, do=8)
    wq8T = t["wq8T"].ap().rearrange("p (do c) -> p do c", do=ndt)
    wkvT = t["wkvT"].ap().rearrange("p (do c) -> p do c", do=ndt)
    woT = t["woT"].ap().rearrange("p (g e) -> p g e", g=NG)
    out2 = t["out2"].ap().rearrange("(st p) e -> p st e", p=P)

    const = ctx.enter_context(tc.tile_pool(name="const", bufs=1))

    def single(shape, name, dtype=BF16):
        return const.tile(shape, dtype, name=name, tag=name)

    wkv_sb = single([P, ndt, 2 * CSL], "wkv_sb")
    wq8_sb = single([P, ndt, CSL], "wq8_sb", FP8)
    wo_sb = single([P, NG, D], "wo_sb")
    xb_sb = single([P, NST, ndt, P], "xb_sb")
    x8_sb = single([P, NSC, ndt, SC], "x8_sb", FP8)
    qf_sb = single([P, NG, S], "qf_sb")
    ones_sb = single([P, 1], "ones_sb")
    bkvT_sb = single([P, NG, P], "bkvT_sb")
    bden_sb = single([P, NG, P], "bden_sb")
    m_sb = single([P, NG, D], "m_sb", FP8 if OUT_FP8 else BF16)
    ksum_sb = single([P, NG], "ksum_sb")
    warm_sb = single([P, 2 * P], "warm_sb")
    # chunk-0 qs lives in the const pool: its den/recip/qs chain runs inside
    # phase A (between the two Q(3) groups) so the first out-proj matmul has
    # its stationary ready the moment the A-phase PSUM pools close
    qs0_sb = single([P, NG, SC], "qs0_sb", FP8 if OUT_FP8 else BF16)

    # ---- PE warm-up: a dozen matmuls on resident data keep the PE busy
    # through the HAM SHORT window during the input-DMA head, so the real
    # matmuls start at 2.4GHz instead of 1.2GHz.
    nc.vector.memset(warm_sb, 1.0)
    with tc.tile_pool(name="ps_warm", bufs=1, space="PSUM") as ps_w:
        warm_ps = ps_w.tile([P, 2 * P], F32, tag="warm_ps")
        for i in range(NWARM):
            nc.tensor.matmul(warm_ps, warm_sb[:, 0:P], warm_sb,
                             start=(i == 0), stop=(i == NWARM - 1))

    # ---- input DMA, need-ordered with per-queue parallelism.  The queues
    # come up staggered (sync ~2.7us after main, scalar ~+1.3us behind its
    # activation-table load, gpsimd ~+2.4us), so the two first-matmul
    # dependencies (wkv half 0 + xb tile 0) ride sync back-to-back.
    h0 = (ndt + 1) // 2
    nc.sync.dma_start(out=wkv_sb[:, 0:h0, :], in_=wkvT[:, 0:h0, :])
    nc.sync.dma_start(out=xb_sb[:, 0, 0:8, :], in_=xbv[:, 0, :, :])
    nc.scalar.dma_start(out=wkv_sb[:, h0:ndt, :], in_=wkvT[:, h0:ndt, :])
    nc.vector.memset(ones_sb, 1.0)
    if use_biases:
        nc.vector.memset(xb_sb[0:1, :, 8, :], 1.0)
        nc.vector.memset(xb_sb[1:P, :, 8, :], 0.0)
        nc.vector.memset(x8_sb[0:1, :, 8, :], 1.0)
        nc.vector.memset(x8_sb[1:P, :, 8, :], 0.0)
    for sti in range(1, 4):
        nc.scalar.dma_start(out=xb_sb[:, sti, 0:8, :], in_=xbv[:, sti, :, :])
    nc.gpsimd.dma_start(out=wq8_sb, in_=wq8T)
    nc.gpsimd.dma_start(out=x8_sb[:, 0, 0:8, :], in_=x8v[:, 0, :, :])
    for sti in range(4, 8):
        nc.gpsimd.dma_start(out=xb_sb[:, sti, 0:8, :], in_=xbv[:, sti, :, :])
    nc.sync.dma_start(out=x8_sb[:, 1, 0:8, :], in_=x8v[:, 1, :, :])
    for sti in range(8, 12):
        nc.sync.dma_start(out=xb_sb[:, sti, 0:8, :], in_=xbv[:, sti, :, :])
    for sti in range(12, NST):
        nc.gpsimd.dma_start(out=xb_sb[:, sti, 0:8, :], in_=xbv[:, sti, :, :])
    for sc in range(2, NSC):
        nc.sync.dma_start(out=x8_sb[:, sc, 0:8, :], in_=x8v[:, sc, :, :])
    nc.gpsimd.dma_start(out=wo_sb, in_=woT)
    nc.gpsimd.memset(bkvT_sb, 0.0)
    nc.gpsimd.memset(bden_sb, 0.0)

    # ---- phase A: K|V projection + feature map + KV^T/ksum accumulation,
    #      Q projection interleaved per s-chunk ----------------------------
    with tc.tile_pool(name="ps_kv", bufs=1, space="PSUM") as ps_kv:
        _phase_a(tc, nc, ps_kv, ndt, xb_sb, x8_sb, wkv_sb, wq8_sb, wo_sb,
                 qf_sb, ones_sb, bkvT_sb, bden_sb, m_sb, ksum_sb, qs0_sb)

    # ---- phase B: normalize q, out[s,e] = sum_g qs_g^T M_g ---------------
    # all den/recip/qs first (they only gate on ksum + qf), then the
    # out-projection streams PE-dense with copies pipelining behind it
    with tc.tile_pool(name="ps_d", bufs=1, space="PSUM") as ps_d, \
         tc.tile_pool(name="ps_o", bufs=7, space="PSUM") as ps_o, \
         tc.tile_pool(name="sb_qs", bufs=4) as sb_qs, \
         tc.tile_pool(name="sb_b", bufs=2) as sb_b:
        qs_tiles = [qs0_sb]
        for sc in range(1, NSC):
            csl = slice(sc * SC, (sc + 1) * SC)
            if OUT_FP8:
                qs = sb_qs.tile([P, NG, SC], FP8, tag="qs8")
            else:
                qs = sb_qs.tile([P, NG, SC], BF16, tag="qs16")
            for g in range(NG):
                d_ps = ps_d.tile([P, SC], F32, tag="d_ps")
                nc.tensor.matmul(d_ps, bden_sb[:, g, :], qf_sb[:, g, csl],
                                 start=True, stop=True)
                rden = sb_b.tile([P, SC], F32, tag=f"rden{g}")
                nc.vector.reciprocal_approx_fast(out=rden, in_=d_ps)
                # qs on gpsimd in 256-col halves: vector/scalar are fully
                # booked with the out-proj PSUM evacuations + reciprocals
                for qh in range(2):
                    hsl = slice(qh * (SC // 2), (qh + 1) * (SC // 2))
                    chsl = slice(sc * SC + qh * (SC // 2),
                                 sc * SC + (qh + 1) * (SC // 2))
                    nc.gpsimd.tensor_tensor(qs[:, g, hsl], qf_sb[:, g, chsl],
                                            rden[:, hsl], ALU.mult)
            qs_tiles.append(qs)
        for sc in range(NSC):
            _emit_outproj(nc, sb_b, ps_o, m_sb, out2, qs_tiles[sc], sc)


def _phase_a(tc, nc, ps_kv, ndt, xb_sb, x8_sb, wkv_sb, wq8_sb, wo_sb, qf_sb,
             ones_sb, bkvT_sb, bden_sb, m_sb, ksum_sb, qs0_sb):
    npair = ndt // 2
    odd = ndt % 2
    kvt_ps = [ps_kv.tile([P, P], F32, name=f"kvt_ps{g}", tag=f"kvt{g}")
              for g in range(NG)]
    # ksum columns for both groups share one bank => ONE accumulation group
    ksum_ps = ps_kv.tile([P, NG], F32, name="ksum_ps", tag="ksum")

    with tc.tile_pool(name="ps_a", bufs=3, space="PSUM") as ps_a, \
         tc.tile_pool(name="ps_q", bufs=2, space="PSUM") as ps_q, \
         tc.tile_pool(name="sb_a", bufs=3) as sb_a:

        def kv_chunk(sc):
            for sti in range(STC):
                st = sc * STC + sti
                # combined K|V projection: [s, 0:256]=K, [s, 256:512]=V
                kvp = ps_a.tile([P, 2 * CSL], F32, tag="kvp")
                for dt in range(ndt):
                    nc.tensor.matmul(
                        kvp, xb_sb[:, st, dt, :], wkv_sb[:, dt, :],
                        start=(dt == 0), stop=(dt == ndt - 1))
                # kf = relu(K) + exp(min(K, 0))   (= elu(K)+1)
                kf = sb_a.tile([P, CSL], BF16, tag="kf")
                m_k = sb_a.tile([P, CSL], BF16, tag="m_k")
                nc.vector.tensor_scalar(m_k, kvp[:, 0:CSL], 0.0, None,
                                        op0=ALU.min)
                nc.scalar.activation(m_k, m_k, AF.Exp)
                nc.vector.scalar_tensor_tensor(
                    kf, kvp[:, 0:CSL], 0.0, m_k, op0=ALU.max, op1=ALU.add)
                v_sb = sb_a.tile([P, CSL], BF16, tag="v_sb")
                nc.scalar.copy(out=v_sb, in_=kvp[:, CSL:2 * CSL])

                # KV^T / ksum accumulation per 128-group:
                #   kvt[cv, ck] += v[s, cv]^T kf[s, ck];  ksum[ck] += kf^T 1
                for g in range(NG):
                    gsl = slice(g * P, (g + 1) * P)
                    nc.tensor.matmul(
                        kvt_ps[g], v_sb[:, gsl], kf[:, gsl],
                        start=(st == 0), stop=(st == NST - 1))
                    nc.tensor.matmul(
                        ksum_ps[:, g:g + 1], kf[:, gsl], ones_sb,
                        start=(st == 0 and g == 0),
                        stop=(st == NST - 1 and g == NG - 1))

        def q_group(sc, g):
            csl = slice(sc * SC, (sc + 1) * SC)
            if True:
                gsl = slice(g * P, (g + 1) * P)
                q_ps = ps_q.tile([P, SC], F32, tag="q_ps")
                if Q_FP8:
                    # DoubleRow: ko pairs of d-tiles, both operands fp8e4
                    for i in range(npair):
                        dsl = slice(2 * i, 2 * i + 2)
                        nc.tensor.matmul(
                            q_ps, wq8_sb[:, dsl, gsl], x8_sb[:, sc, dsl, :],
                            start=(i == 0), stop=(i == npair - 1 and not odd),
                            perf_mode=DR)
                    if odd:
                        nc.tensor.matmul(
                            q_ps, wq8_sb[:, ndt - 1, gsl],
                            x8_sb[:, sc, ndt - 1, :], start=False, stop=True)
                else:
                    for dt in range(ndt):
                        nc.tensor.matmul(
                            q_ps, wq8_sb[:, dt, gsl], x8_sb[:, sc, dt, :],
                            start=(dt == 0), stop=(dt == ndt - 1))
                m_q = sb_a.tile([P, SC], BF16, tag="m_q")
                nc.vector.tensor_scalar(m_q, q_ps, 0.0, None, op0=ALU.min)
                nc.scalar.activation(m_q, m_q, AF.Exp)
                nc.vector.scalar_tensor_tensor(
                    qf_sb[:, g, csl], q_ps, 0.0, m_q,
                    op0=ALU.max, op1=ALU.add)

        def q_chunk(sc):
            q_group(sc, 0)
            q_group(sc, 1)

        # the LAST chunk runs KV first, then the boundary extraction is
        # emitted BEFORE the final Q projection: the copies land in the
        # vector/scalar/gpsimd queues ahead of Q(3)'s feature ops and run
        # during Q(3)'s PE time.  Chunk 0's den/recip/qs chain is woven
        # between Q(3)'s two groups so the out-projection's first stationary
        # (qs0) is ready the moment phase B opens.
        kv_chunk(0); q_chunk(0)
        kv_chunk(1); q_chunk(1)
        kv_chunk(2); q_chunk(2)
        kv_chunk(3)
        # bkvT[g][cv, ck] = KV^T for head(cv)==head(ck) else 0
        # bden[g][ck', ck] = ksum[ck']*s for head(ck')==head(ck) else 0
        # (s = 2^-17 when OUT_FP8 so rden = 2^17/den lands qs in fp8 range).
        # ksum -> SBUF once (vector reads PSUM), then the broadcast fills
        # run on gpsimd so the A-end vector queue (q3 feature ops) doesn't
        # delay the first den matmul.
        nc.vector.tensor_scalar(ksum_sb, ksum_ps,
                                1.0 / QS_SCALE if OUT_FP8 else 1.0, None,
                                op0=ALU.mult)
        for g in range(NG):
            for hb in range(2):
                hsl = slice(hb * 64, (hb + 1) * 64)
                nc.scalar.copy(out=bkvT_sb[hsl, g, hsl],
                               in_=kvt_ps[g][hsl, hsl])
                nc.gpsimd.tensor_copy(
                    out=bden_sb[hsl, g, hsl],
                    in_=ksum_sb[hsl, g:g + 1].to_broadcast((64, 64)))
        q_group(3, 0)
        # chunk-0 denominator chain, PE-covered by Q(3) g1 + the M matmuls
        for g in range(NG):
            d_ps = ps_q.tile([P, SC], F32, tag="q_ps")
            nc.tensor.matmul(d_ps, bden_sb[:, g, :], qf_sb[:, g, 0:SC],
                             start=True, stop=True)
            rden0 = sb_a.tile([P, SC], F32, tag=f"rden0_{g}")
            nc.vector.reciprocal_approx_fast(out=rden0, in_=d_ps)
            for qh in range(2):
                hsl = slice(qh * (SC // 2), (qh + 1) * (SC // 2))
                nc.gpsimd.tensor_tensor(qs0_sb[:, g, hsl],
                                        qf_sb[:, g, hsl], rden0[:, hsl],
                                        ALU.mult)
        q_group(3, 1)

    with tc.tile_pool(name="ps_m", bufs=2, space="PSUM") as ps_m:
        for g in range(NG):
            for eh in range(2):
                esl = slice(eh * EH, (eh + 1) * EH)
                m_ps = ps_m.tile([P, EH], F32, tag="m_ps")
                nc.tensor.matmul(m_ps, bkvT_sb[:, g, :], wo_sb[:, g, esl],
                                 start=True, stop=True)
                scale = M_SCALE if OUT_FP8 else 1.0
                if eh == 0:
                    nc.scalar.activation(m_sb[:, g, esl], m_ps, AF.Copy,
                                         scale=scale)
                else:
                    nc.vector.tensor_scalar(m_sb[:, g, esl], m_ps, scale,
                                            None, op0=ALU.mult)


def _emit_outproj(nc, sb_b, ps_o, m_sb, out2, qs, sc):
    # per-s-tile 256KB output DMAs alternating sync/gpsimd: each departs
    # after only its own two PSUM evacuations (subtile deps), keeping the
    # output stream dense through phase B and the post-compute drain short
    for half in range(2):
        o_sb = sb_b.tile([P, STC // 2, D], BF16, tag=f"o_sb{half}")
        for hi in range(STC // 2):
            sti = half * 2 + hi
            tsl = slice(sti * P, (sti + 1) * P)
            for eh in range(2):
                esl = slice(eh * EH, (eh + 1) * EH)
                o_ps = ps_o.tile([P, EH], F32, tag="o_ps")
                if OUT_FP8:
                    # one DoubleRow MM sums both head groups via the ko dim
                    nc.tensor.matmul(o_ps, qs[:, :, tsl], m_sb[:, :, esl],
                                     start=True, stop=True, perf_mode=DR)
                else:
                    for g in range(NG):
                        nc.tensor.matmul(
                            o_ps, qs[:, g, tsl], m_sb[:, g, esl],
                            start=(g == 0), stop=(g == NG - 1))
                # PSUM evacuation balanced 4/4 over scalar/vector per chunk
                oscale = O_SCALE if OUT_FP8 else 1.0
                if eh == 0:
                    nc.scalar.activation(o_sb[:, hi, esl], o_ps, AF.Copy,
                                         scale=oscale)
                else:
                    nc.vector.tensor_scalar(o_sb[:, hi, esl], o_ps, oscale,
                                            None, op0=ALU.mult)
            gsti = sc * STC + sti
            eng = nc.sync if gsti % 2 == 0 else nc.gpsimd
            eng.dma_start(out=out2[:, gsti:gsti + 1, :],
                          in_=o_sb[:, hi:hi + 1, :])


def build_nc(use_biases):
    nc = bacc.Bacc("TRN2", target_bir_lowering=False, debug=False)
    ndt = 9 if use_biases else 8
    t = {}
    t["xb"] = nc.dram_tensor("xb", [NST * P, 8 * P], BF16,
                             kind="ExternalInput")
    t["x8"] = nc.dram_tensor("x8", [NSC * P, 8 * SC], FP8,
                             kind="ExternalInput")
    t["wq8T"] = nc.dram_tensor("wq8T", [P, ndt * CSL], FP8,
                               kind="ExternalInput")
    t["wkvT"] = nc.dram_tensor("wkvT", [P, ndt * 2 * CSL], BF16,
                               kind="ExternalInput")
    t["woT"] = nc.dram_tensor("woT", [P, NG * D], BF16, kind="ExternalInput")
    t["out2"] = nc.dram_tensor("out2", [S, D], BF16, kind="ExternalOutput")

    with tile.TileContext(nc) as tc:
        with ExitStack() as ctx:
            _build_kernel_body(ctx, tc, t, use_biases)
    nc.compile()
    return nc


def _get_nc(use_biases):
    key = ("nc", use_biases)
    if key not in _CACHE:
        _CACHE[key] = build_nc(use_biases)
    return _CACHE[key]


def make_in_maps(x, wq, bq, wk, bk, wv, bv, wo, bo, use_biases=None):
    """Shard the full inputs into the 8 per-core input maps."""
    f = lambda a: np.asarray(a, dtype=np.float32)
    x, wq, bq, wk, bk = f(x), f(wq), f(bq), f(wk), f(bk)
    wv, bv, wo, bo = f(wv), f(bv), f(wo), f(bo)
    if use_biases is None:
        # bo is applied host-side in unshard(); bq/bk/bv need the in-GEMM path
        use_biases = any(np.any(b) for b in (bq, bk, bv))
    bf = lambda a: np.ascontiguousarray(a).astype(NPBF)
    f8 = lambda a: np.ascontiguousarray(a).astype(NPF8)
    ndt = 9 if use_biases else 8

    def tiled_w(wT):
        # [ndt*128, c] -> [128, ndt*c]: per-partition contiguous weight rows
        c = wT.shape[1]
        return wT.reshape(ndt, P, c).transpose(1, 0, 2).reshape(P, ndt * c)

    xparts = {}
    for b in range(BATCH):
        xt = np.ascontiguousarray(x[b].T)            # [D, S]
        v4 = xt.reshape(8, P, NST, P)
        # tile-major: row (st*P + p) = [do, s] -- 2KB contiguous per row
        xb_t = v4.transpose(2, 1, 0, 3).reshape(NST * P, 8 * P)
        v5 = xt.reshape(8, P, NSC, SC)
        # chunk-major: row (sc*P + p) = [do, s] -- 4KB contiguous per row
        x8_t = v5.transpose(2, 1, 0, 3).reshape(NSC * P, 8 * SC)
        xparts[b] = (bf(xb_t), f8(x8_t))

    in_maps = []
    for cid in range(8):
        b, hg = divmod(cid, 4)
        hs = slice(hg * CSL, (hg + 1) * CSL)
        wkvT = np.concatenate([wk[hs, :].T, wv[hs, :].T], axis=1)
        wqT = wq[hs, :].T
        if use_biases:
            # bias row at row D (multiplied by the on-chip ones row), zero
            # padding to the 9*128 augmented contraction size
            wkvT = np.concatenate(
                [wkvT, np.concatenate([bk[hs], bv[hs]])[None, :],
                 np.zeros((P - 1, 2 * CSL), np.float32)], axis=0)
            wqT = np.concatenate(
                [wqT, bq[hs][None, :], np.zeros((P - 1, CSL), np.float32)],
                axis=0)
        woT = wo[:, hs].T.reshape(NG, P, D).transpose(1, 0, 2).reshape(P, -1)
        m = {
            "xb": xparts[b][0],
            "x8": xparts[b][1],
            "wq8T": f8(tiled_w(wqT)),
            "wkvT": bf(tiled_w(wkvT)),
            "woT": bf(woT),
        }
        in_maps.append(m)
    return in_maps, use_biases


def unshard(results, bo=None):
    """Sum head-group partials per batch (tensor-parallel unshard)."""
    out = np.zeros((BATCH, S, D), np.float32)
    for cid in range(8):
        b = cid // 4
        out[b] += np.asarray(results[cid]["out2"]).astype(np.float32)
    if bo is not None:
        bo = np.asarray(bo, np.float32)
        if np.any(bo):
            out += bo[None, None, :]
    return out


def kernel(x, wq, bq, wk, bk, wv, bv, wo, bo):
    in_maps, use_biases = make_in_maps(x, wq, bq, wk, bk, wv, bv, wo, bo)
    nc = _get_nc(use_biases)
    res = bass_utils.run_bass_kernel_spmd(nc, in_maps, core_ids=list(range(8)))
    return unshard(res.results, bo=bo)


# revision 20
# speedup vs baseline: 1.0767x; 1.0767x over previous
"""Trainium2 Bass kernel for ConvexLinearAttention (elu(x)+1 linear attention).

Full-input contract: kernel(**inputs) takes the unsharded tensors
(x [2,2048,1024], wq/wk/wv/wo [1024,1024], bq/bk/bv/bo [1024]) and returns the
full output [2,2048,1024].

Sharding (8 cores): data-parallel over batch (2) x head-group-parallel (4 groups
of 4 heads).  Each core projects only its 256-wide head slice, runs the
linearized attention (attended = qf @ (kf^T V) / (qf @ sum(kf)) -- an exact
refactoring of the dense normalized scores), and emits a partial output
projection in natural [s, e] layout.  The host sums the 4 head-group partials
per batch.

Precision plan (tolerance 2e-2, bf16 baseline err ~4e-3):
  - K|V projection stays bf16: V weight-quantization error is common-mode
    across sequence positions (x_bar @ dW) and does NOT average out in the
    attention mean -- fp8 V alone measured 3.3e-2.
  - Q projection: fp8e4 DoubleRow (wq8 stationary [128,2,128], x8 moving
    [128,2,512]).  Normalization cancels most q-side noise: 7.8e-3 alone.
  - Out projection: fp8e4 DoubleRow, one MM per (s-tile, e-half) pairing the
    two 128-wide head groups in the ko dim.  qs and M have disjoint dynamic
    ranges (qs ~ 7e-6, M ~ 1e2), so split scales: qs*2^17 (folded into the
    scaled ksum feeding the denominator matmul) and M*2^-6 (folded into the
    M PSUM evacuation); the out PSUM evacuation multiplies by 2^-11.
    Q+out fp8 together measured 1.7e-2 in simulation vs the 2e-2 gate.

DMA plan (per-queue throughput is descriptor-paced: ~35ns + ~60ns/KB per
descriptor, so both the dram AND sbuf side of every transfer must be
contiguous in 2KB+ runs):
  - xb (bf16 x for the K|V stationary) is tile-major on BOTH sides:
    dram rows (st p) hold [do s] 2KB runs, sbuf is [P, NST, ndt, P].
  - x8 (fp8 x, Q moving operand) is chunk-major: dram rows (sc p) hold
    [do s] 4KB runs, sbuf is [P, NSC, ndt, SC].
  - wkv streams as 2 halves on different queues so the dt-ordered matmul
    consumption starts after ~512KB.
  - Input triggers ride only sync/gpsimd: a dma_start occupies a slot in its
    engine's instruction stream and can block behind ring semaphores, so the
    scalar/vector queues (which carry the latency-critical feature ops) get
    no input DMAs after the first four.
  - 12 warm-up matmuls on resident data run during the input head so the PE
    HAM clock gate is already 8/8 when real work starts.
"""

from contextlib import ExitStack

import numpy as np
import ml_dtypes

import concourse.bass as bass
import concourse.mybir as mybir
import concourse.tile as tile
from concourse import bacc, bass_utils

F32 = mybir.dt.float32
BF16 = mybir.dt.bfloat16
FP8 = mybir.dt.float8e4
AF = mybir.ActivationFunctionType
ALU = mybir.AluOpType
DR = mybir.MatmulPerfMode.DoubleRow

NPBF = ml_dtypes.bfloat16
NPF8 = ml_dtypes.float8_e4m3  # IEEE-style e4m3 (max 240) = TRN float8e4

D = 1024          # model dim
S = 2048          # sequence length
BATCH = 2
CSL = 256         # head-slice width per core (4 heads x 64)
NG = 2            # 128-wide c-groups per core
P = 128
NST = S // P      # 16 s-tiles
SC = 512          # s-chunk
NSC = S // SC     # 4 s-chunks
STC = SC // P     # 4 s-tiles per chunk
EH = 512          # e-half width for out-proj rhs

Q_FP8 = True      # fp8 DoubleRow Q projection
# fp8 DoubleRow out projection (split scales).  Measured SLOWER than bf16:
# the single-MM-per-bank DoubleRow stream pays a serialized 256-col
# LDWEIGHTS per matmul (376ns vs 216) plus inter-MM gaps that let the HAM
# clock gate oscillate; the bf16 2-MM accumulation groups stream at 216ns
# with FWL-hidden weight loads.  Kept as a switch for reference.
OUT_FP8 = False
QS_SCALE = 2.0 ** 17   # qs -> qs * QS_SCALE (folded into scaled ksum)
M_SCALE = 2.0 ** -6    # M  -> M * M_SCALE  (folded into M PSUM evac)
O_SCALE = 1.0 / (QS_SCALE * M_SCALE)  # out evac multiplier (2^-11)
NWARM = 16

_CACHE: dict = {}


def install_ntff_hook_shim():
    """Provide ``antenv.axon_hooks`` when the image ships only the antenv stub.

    concourse.bass_utils imports it unconditionally on the axon trace path;
    without this shim trace=True (or BASS_TRACE=1) crashes.  Registers the real
    ctypes NTFF hook when the axon .so is present, else a None-returning stub
    so tracing degrades gracefully.
    """
    import os
    import sys
    import types

    if "antenv.axon_hooks" in sys.modules:
        return
    try:
        import antenv
        import antenv.axon_hooks  # noqa: F401
        return  # real module exists
    except ImportError:
        pass
    mod = types.ModuleType("antenv.axon_hooks")
    state: dict = {"h": None}
    mod.set_axon_ntff_profile_hook = lambda h: state.__setitem__("h", h)
    mod.get_axon_ntff_profile_hook = lambda: state.get("h")
    sys.modules["antenv.axon_hooks"] = mod
    antenv.axon_hooks = mod
    so_path = "/opt/axon/libaxon_pjrt.so"
    if os.path.exists(so_path):
        try:
            from trn_agent_boot.trn_boot import _ntff_profile_via_ctypes

            state["h"] = _ntff_profile_via_ctypes(so_path)
        except Exception:
            pass


def _build_kernel_body(ctx: ExitStack, tc: tile.TileContext, t, use_biases):
    nc = tc.nc
    # with biases, a 9th d-tile (ones row 0, zeros elsewhere) multiplies the
    # bias row appended to the weight matrices: exact bias add inside the GEMM
    ndt = 9 if use_biases else 8

    xbv = t["xb"].ap().rearrange("(st p) (do s) -> p st do s", p=P, do=8)
    x8v = t["x8"].ap().rearrange("(sc p) (do s) -> p sc do s", p=P, do=8)
    wq8T = t["wq8T"].ap().rearrange("p (do c) -> p do c", do=ndt)
    wkvT = t["wkvT"].ap().rearrange("p (do c) -> p do c", do=ndt)
    woT = t["woT"].ap().rearrange("p (g e) -> p g e", g=NG)
    out2 = t["out2"].ap().rearrange("(st p) e -> p st e", p=P)

    const = ctx.enter_context(tc.tile_pool(name="const", bufs=1))

    def single(shape, name, dtype=BF16):
        return const.tile(shape, dtype, name=name, tag=name)

    wkv_sb = single([P, ndt, 2 * CSL], "wkv_sb")
    wq8_sb = single([P, ndt, CSL], "wq8_sb", FP8)
    wo_sb = single([P, NG, D], "wo_sb")
    xb_sb = single([P, NST, ndt, P], "xb_sb")
    x8_sb = single([P, NSC, ndt, SC], "x8_sb", FP8)
    qf_sb = single([P, NG, S], "qf_sb")
    ones_sb = single([P, 1], "ones_sb")
    bkvT_sb = single([P, NG, P], "bkvT_sb")
    bden_sb = single([P, NG, P], "bden_sb")
    m_sb = single([P, NG, D], "m_sb", FP8 if OUT_FP8 else BF16)
    ksum_sb = single([P, NG], "ksum_sb")
    warm_sb = single([P, 2 * P], "warm_sb")
    # chunk-0 qs lives in the const pool: its den/recip/qs chain runs inside
    # phase A (between the two Q(3) groups) so the first out-proj matmul has
    # its stationary ready the moment the A-phase PSUM pools close
    qs0_sb = single([P, NG, SC], "qs0_sb", FP8 if OUT_FP8 else BF16)

    # ---- PE warm-up: a dozen matmuls on resident data keep the PE busy
    # through the HAM SHORT window during the input-DMA head, so the real
    # matmuls start at 2.4GHz instead of 1.2GHz.
    nc.vector.memset(warm_sb, 1.0)
    with tc.tile_pool(name="ps_warm", bufs=1, space="PSUM") as ps_w:
        warm_ps = ps_w.tile([P, 2 * P], F32, tag="warm_ps")
        for i in range(NWARM):
            nc.tensor.matmul(warm_ps, warm_sb[:, 0:P], warm_sb,
                             start=(i == 0), stop=(i == NWARM - 1))

    # ---- input DMA, need-ordered with per-queue parallelism.  The three
    # first-matmul dependencies lead the three DMA-capable queues so they
    # stream concurrently: wkv half 0 on sync (earliest to come up), xb
    # tile 0 on scalar, wkv half 1 on gpsimd.  Bulk trails in need order;
    # every descriptor is a 2-4KB contiguous run on both sides.
    h0 = (ndt + 1) // 2
    nc.sync.dma_start(out=wkv_sb[:, 0:h0, :], in_=wkvT[:, 0:h0, :])
    nc.scalar.dma_start(out=xb_sb[:, 0, 0:8, :], in_=xbv[:, 0, :, :])
    nc.gpsimd.dma_start(out=wkv_sb[:, h0:ndt, :], in_=wkvT[:, h0:ndt, :])
    nc.vector.memset(ones_sb, 1.0)
    if use_biases:
        nc.vector.memset(xb_sb[0:1, :, 8, :], 1.0)
        nc.vector.memset(xb_sb[1:P, :, 8, :], 0.0)
        nc.vector.memset(x8_sb[0:1, :, 8, :], 1.0)
        nc.vector.memset(x8_sb[1:P, :, 8, :], 0.0)
    nc.scalar.dma_start(out=xb_sb[:, 1, 0:8, :], in_=xbv[:, 1, :, :])
    nc.scalar.dma_start(out=wq8_sb, in_=wq8T)
    for sti in range(2, 4):
        nc.sync.dma_start(out=xb_sb[:, sti, 0:8, :], in_=xbv[:, sti, :, :])
    nc.gpsimd.dma_start(out=x8_sb[:, 0, 0:8, :], in_=x8v[:, 0, :, :])
    for sti in range(4, 8):
        nc.gpsimd.dma_start(out=xb_sb[:, sti, 0:8, :], in_=xbv[:, sti, :, :])
    nc.gpsimd.dma_start(out=x8_sb[:, 1, 0:8, :], in_=x8v[:, 1, :, :])
    for sti in range(8, 12):
        nc.gpsimd.dma_start(out=xb_sb[:, sti, 0:8, :], in_=xbv[:, sti, :, :])
    nc.gpsimd.dma_start(out=x8_sb[:, 2, 0:8, :], in_=x8v[:, 2, :, :])
    for sti in range(12, NST):
        nc.gpsimd.dma_start(out=xb_sb[:, sti, 0:8, :], in_=xbv[:, sti, :, :])
    nc.gpsimd.dma_start(out=x8_sb[:, 3, 0:8, :], in_=x8v[:, 3, :, :])
    nc.gpsimd.dma_start(out=wo_sb, in_=woT)
    nc.gpsimd.memset(bkvT_sb, 0.0)
    nc.gpsimd.memset(bden_sb, 0.0)

    # ---- phase A: K|V projection + feature map + KV^T/ksum accumulation,
    #      Q projection interleaved per s-chunk ----------------------------
    with tc.tile_pool(name="ps_kv", bufs=1, space="PSUM") as ps_kv:
        _phase_a(tc, nc, ps_kv, ndt, xb_sb, x8_sb, wkv_sb, wq8_sb, wo_sb,
                 qf_sb, ones_sb, bkvT_sb, bden_sb, m_sb, ksum_sb, qs0_sb)

    # ---- phase B: normalize q, out[s,e] = sum_g qs_g^T M_g ---------------
    # all den/recip/qs first (they only gate on ksum + qf), then the
    # out-projection streams PE-dense with copies pipelining behind it
    with tc.tile_pool(name="ps_d", bufs=1, space="PSUM") as ps_d, \
         tc.tile_pool(name="ps_o", bufs=7, space="PSUM") as ps_o, \
         tc.tile_pool(name="sb_qs", bufs=4) as sb_qs, \
         tc.tile_pool(name="sb_b", bufs=2) as sb_b:
        qs_tiles = [qs0_sb]
        for sc in range(1, NSC):
            csl = slice(sc * SC, (sc + 1) * SC)
            if OUT_FP8:
                qs = sb_qs.tile([P, NG, SC], FP8, tag="qs8")
            else:
                qs = sb_qs.tile([P, NG, SC], BF16, tag="qs16")
            for g in range(NG):
                d_ps = ps_d.tile([P, SC], F32, tag="d_ps")
                nc.tensor.matmul(d_ps, bden_sb[:, g, :], qf_sb[:, g, csl],
                                 start=True, stop=True)
                rden = sb_b.tile([P, SC], F32, tag=f"rden{g}")
                nc.vector.reciprocal_approx_fast(out=rden, in_=d_ps)
                # qs on gpsimd in 256-col halves: vector/scalar are fully
                # booked with the out-proj PSUM evacuations + reciprocals
                for qh in range(2):
                    hsl = slice(qh * (SC // 2), (qh + 1) * (SC // 2))
                    chsl = slice(sc * SC + qh * (SC // 2),
                                 sc * SC + (qh + 1) * (SC // 2))
                    nc.gpsimd.tensor_tensor(qs[:, g, hsl], qf_sb[:, g, chsl],
                                            rden[:, hsl], ALU.mult)
            qs_tiles.append(qs)
        for sc in range(NSC):
            _emit_outproj(nc, sb_b, ps_o, m_sb, out2, qs_tiles[sc], sc)


def _phase_a(tc, nc, ps_kv, ndt, xb_sb, x8_sb, wkv_sb, wq8_sb, wo_sb, qf_sb,
             ones_sb, bkvT_sb, bden_sb, m_sb, ksum_sb, qs0_sb):
    npair = ndt // 2
    odd = ndt % 2
    kvt_ps = [ps_kv.tile([P, P], F32, name=f"kvt_ps{g}", tag=f"kvt{g}")
              for g in range(NG)]
    # ksum columns for both groups share one bank => ONE accumulation group
    ksum_ps = ps_kv.tile([P, NG], F32, name="ksum_ps", tag="ksum")

    with tc.tile_pool(name="ps_a", bufs=3, space="PSUM") as ps_a, \
         tc.tile_pool(name="ps_q", bufs=2, space="PSUM") as ps_q, \
         tc.tile_pool(name="sb_a", bufs=3) as sb_a:

        def kv_chunk(sc):
            for sti in range(STC):
                st = sc * STC + sti
                # combined K|V projection: [s, 0:256]=K, [s, 256:512]=V
                kvp = ps_a.tile([P, 2 * CSL], F32, tag="kvp")
                for dt in range(ndt):
                    nc.tensor.matmul(
                        kvp, xb_sb[:, st, dt, :], wkv_sb[:, dt, :],
                        start=(dt == 0), stop=(dt == ndt - 1))
                # kf = relu(K) + exp(min(K, 0))   (= elu(K)+1)
                kf = sb_a.tile([P, CSL], BF16, tag="kf")
                m_k = sb_a.tile([P, CSL], BF16, tag="m_k")
                nc.vector.tensor_scalar(m_k, kvp[:, 0:CSL], 0.0, None,
                                        op0=ALU.min)
                nc.scalar.activation(m_k, m_k, AF.Exp)
                nc.vector.scalar_tensor_tensor(
                    kf, kvp[:, 0:CSL], 0.0, m_k, op0=ALU.max, op1=ALU.add)
                v_sb = sb_a.tile([P, CSL], BF16, tag="v_sb")
                nc.scalar.copy(out=v_sb, in_=kvp[:, CSL:2 * CSL])

                # KV^T / ksum accumulation per 128-group:
                #   kvt[cv, ck] += v[s, cv]^T kf[s, ck];  ksum[ck] += kf^T 1
                for g in range(NG):
                    gsl = slice(g * P, (g + 1) * P)
                    nc.tensor.matmul(
                        kvt_ps[g], v_sb[:, gsl], kf[:, gsl],
                        start=(st == 0), stop=(st == NST - 1))
                    nc.tensor.matmul(
                        ksum_ps[:, g:g + 1], kf[:, gsl], ones_sb,
                        start=(st == 0 and g == 0),
                        stop=(st == NST - 1 and g == NG - 1))

        def q_group(sc, g):
            csl = slice(sc * SC, (sc + 1) * SC)
            if True:
                gsl = slice(g * P, (g + 1) * P)
                q_ps = ps_q.tile([P, SC], F32, tag="q_ps")
                if Q_FP8:
                    # DoubleRow: ko pairs of d-tiles, both operands fp8e4
                    for i in range(npair):
                        dsl = slice(2 * i, 2 * i + 2)
                        nc.tensor.matmul(
                            q_ps, wq8_sb[:, dsl, gsl], x8_sb[:, sc, dsl, :],
                            start=(i == 0), stop=(i == npair - 1 and not odd),
                            perf_mode=DR)
                    if odd:
                        nc.tensor.matmul(
                            q_ps, wq8_sb[:, ndt - 1, gsl],
                            x8_sb[:, sc, ndt - 1, :], start=False, stop=True)
                else:
                    for dt in range(ndt):
                        nc.tensor.matmul(
                            q_ps, wq8_sb[:, dt, gsl], x8_sb[:, sc, dt, :],
                            start=(dt == 0), stop=(dt == ndt - 1))
                m_q = sb_a.tile([P, SC], BF16, tag="m_q")
                nc.vector.tensor_scalar(m_q, q_ps, 0.0, None, op0=ALU.min)
                nc.scalar.activation(m_q, m_q, AF.Exp)
                nc.vector.scalar_tensor_tensor(
                    qf_sb[:, g, csl], q_ps, 0.0, m_q,
                    op0=ALU.max, op1=ALU.add)

        def q_chunk(sc):
            q_group(sc, 0)
            q_group(sc, 1)

        # the LAST chunk runs KV first, then the boundary extraction is
        # emitted BEFORE the final Q projection: the copies land in the
        # vector/scalar/gpsimd queues ahead of Q(3)'s feature ops and run
        # during Q(3)'s PE time.  Chunk 0's den/recip/qs chain is woven
        # between Q(3)'s two groups so the out-projection's first stationary
        # (qs0) is ready the moment phase B opens.
        kv_chunk(0); q_chunk(0)
        kv_chunk(1); q_chunk(1)
        kv_chunk(2); q_chunk(2)
        kv_chunk(3)
        # bkvT[g][cv, ck] = KV^T for head(cv)==head(ck) else 0
        # bden[g][ck', ck] = ksum[ck']*s for head(ck')==head(ck) else 0
        # (s = 2^-17 when OUT_FP8 so rden = 2^17/den lands qs in fp8 range).
        # ksum -> SBUF once (vector reads PSUM), then the broadcast fills
        # run on gpsimd so the A-end vector queue (q3 feature ops) doesn't
        # delay the first den matmul.
        nc.vector.tensor_scalar(ksum_sb, ksum_ps,
                                1.0 / QS_SCALE if OUT_FP8 else 1.0, None,
                                op0=ALU.mult)
        for g in range(NG):
            for hb in range(2):
                hsl = slice(hb * 64, (hb + 1) * 64)
                nc.scalar.copy(out=bkvT_sb[hsl, g, hsl],
                               in_=kvt_ps[g][hsl, hsl])
                nc.gpsimd.tensor_copy(
                    out=bden_sb[hsl, g, hsl],
                    in_=ksum_sb[hsl, g:g + 1].to_broadcast((64, 64)))
        q_group(3, 0)
        # chunk-0 denominator chain, PE-covered by Q(3) g1 + the M matmuls
        for g in range(NG):
            d_ps = ps_q.tile([P, SC], F32, tag="q_ps")
            nc.tensor.matmul(d_ps, bden_sb[:, g, :], qf_sb[:, g, 0:SC],
                             start=True, stop=True)
            rden0 = sb_a.tile([P, SC], F32, tag=f"rden0_{g}")
            nc.vector.reciprocal_approx_fast(out=rden0, in_=d_ps)
            for qh in range(2):
                hsl = slice(qh * (SC // 2), (qh + 1) * (SC // 2))
                nc.gpsimd.tensor_tensor(qs0_sb[:, g, hsl],
                                        qf_sb[:, g, hsl], rden0[:, hsl],
                                        ALU.mult)
        q_group(3, 1)

    with tc.tile_pool(name="ps_m", bufs=2, space="PSUM") as ps_m:
        for g in range(NG):
            for eh in range(2):
                esl = slice(eh * EH, (eh + 1) * EH)
                m_ps = ps_m.tile([P, EH], F32, tag="m_ps")
                nc.tensor.matmul(m_ps, bkvT_sb[:, g, :], wo_sb[:, g, esl],
                                 start=True, stop=True)
                scale = M_SCALE if OUT_FP8 else 1.0
                if eh == 0:
                    nc.scalar.activation(m_sb[:, g, esl], m_ps, AF.Copy,
                                         scale=scale)
                else:
                    nc.vector.tensor_scalar(m_sb[:, g, esl], m_ps, scale,
                                            None, op0=ALU.mult)


def _emit_outproj(nc, sb_b, ps_o, m_sb, out2, qs, sc):
    # per-s-tile 256KB output DMAs alternating sync/gpsimd: each departs
    # after only its own two PSUM evacuations (subtile deps), keeping the
    # output stream dense through phase B and the post-compute drain short
    for half in range(2):
        o_sb = sb_b.tile([P, STC // 2, D], BF16, tag=f"o_sb{half}")
        for hi in range(STC // 2):
            sti = half * 2 + hi
            tsl = slice(sti * P, (sti + 1) * P)
            for eh in range(2):
                esl = slice(eh * EH, (eh + 1) * EH)
                o_ps = ps_o.tile([P, EH], F32, tag="o_ps")
                if OUT_FP8:
                    # one DoubleRow MM sums both head groups via the ko dim
                    nc.tensor.matmul(o_ps, qs[:, :, tsl], m_sb[:, :, esl],
                                     start=True, stop=True, perf_mode=DR)
                else:
                    for g in range(NG):
                        nc.tensor.matmul(
                            o_ps, qs[:, g, tsl], m_sb[:, g, esl],
                            start=(g == 0), stop=(g == NG - 1))
                # PSUM evacuation balanced 4/4 over scalar/vector per chunk
                oscale = O_SCALE if OUT_FP8 else 1.0
                if eh == 0:
                    nc.scalar.activation(o_sb[:, hi, esl], o_ps, AF.Copy,
                                         scale=oscale)
                else:
                    nc.vector.tensor_scalar(o_sb[:, hi, esl], o_ps, oscale,
                                            None, op0=ALU.mult)
            # outputs ride sync/scalar: gpsimd's queue carries the qs
            # multiplies, and a DMA trigger's ring-semaphore wait would
            # stall them (and with them the out-proj matmuls)
            gsti = sc * STC + sti
            eng = nc.sync if gsti % 2 == 0 else nc.scalar
            eng.dma_start(out=out2[:, gsti:gsti + 1, :],
                          in_=o_sb[:, hi:hi + 1, :])


def build_nc(use_biases):
    nc = bacc.Bacc("TRN2", target_bir_lowering=False, debug=False)
    ndt = 9 if use_biases else 8
    t = {}
    t["xb"] = nc.dram_tensor("xb", [NST * P, 8 * P], BF16,
                             kind="ExternalInput")
    t["x8"] = nc.dram_tensor("x8", [NSC * P, 8 * SC], FP8,
                             kind="ExternalInput")
    t["wq8T"] = nc.dram_tensor("wq8T", [P, ndt * CSL], FP8,
                               kind="ExternalInput")
    t["wkvT"] = nc.dram_tensor("wkvT", [P, ndt * 2 * CSL], BF16,
                               kind="ExternalInput")
    t["woT"] = nc.dram_tensor("woT", [P, NG * D], BF16, kind="ExternalInput")
    t["out2"] = nc.dram_tensor("out2", [S, D], BF16, kind="ExternalOutput")

    with tile.TileContext(nc) as tc:
        with ExitStack() as ctx:
            _build_kernel_body(ctx, tc, t, use_biases)
    nc.compile()
    return nc


def _get_nc(use_biases):
    key = ("nc", use_biases)
    if key not in _CACHE:
        _CACHE[key] = build_nc(use_biases)
    return _CACHE[key]


def make_in_maps(x, wq, bq, wk, bk, wv, bv, wo, bo, use_biases=None):
    """Shard the full inputs into the 8 per-core input maps."""
    f = lambda a: np.asarray(a, dtype=np.float32)
    x, wq, bq, wk, bk = f(x), f(wq), f(bq), f(wk), f(bk)
    wv, bv, wo, bo = f(wv), f(bv), f(wo), f(bo)
    if use_biases is None:
        # bo is applied host-side in unshard(); bq/bk/bv need the in-GEMM path
        use_biases = any(np.any(b) for b in (bq, bk, bv))
    bf = lambda a: np.ascontiguousarray(a).astype(NPBF)
    f8 = lambda a: np.ascontiguousarray(a).astype(NPF8)
    ndt = 9 if use_biases else 8

    def tiled_w(wT):
        # [ndt*128, c] -> [128, ndt*c]: per-partition contiguous weight rows
        c = wT.shape[1]
        return wT.reshape(ndt, P, c).transpose(1, 0, 2).reshape(P, ndt * c)

    xparts = {}
    for b in range(BATCH):
        xt = np.ascontiguousarray(x[b].T)            # [D, S]
        v4 = xt.reshape(8, P, NST, P)
        # tile-major: row (st*P + p) = [do, s] -- 2KB contiguous per row
        xb_t = v4.transpose(2, 1, 0, 3).reshape(NST * P, 8 * P)
        v5 = xt.reshape(8, P, NSC, SC)
        # chunk-major: row (sc*P + p) = [do, s] -- 4KB contiguous per row
        x8_t = v5.transpose(2, 1, 0, 3).reshape(NSC * P, 8 * SC)
        xparts[b] = (bf(xb_t), f8(x8_t))

    in_maps = []
    for cid in range(8):
        b, hg = divmod(cid, 4)
        hs = slice(hg * CSL, (hg + 1) * CSL)
        wkvT = np.concatenate([wk[hs, :].T, wv[hs, :].T], axis=1)
        wqT = wq[hs, :].T
        if use_biases:
            # bias row at row D (multiplied by the on-chip ones row), zero
            # padding to the 9*128 augmented contraction size
            wkvT = np.concatenate(
                [wkvT, np.concatenate([bk[hs], bv[hs]])[None, :],
                 np.zeros((P - 1, 2 * CSL), np.float32)], axis=0)
            wqT = np.concatenate(
                [wqT, bq[hs][None, :], np.zeros((P - 1, CSL), np.float32)],
                axis=0)
        woT = wo[:, hs].T.reshape(NG, P, D).transpose(1, 0, 2).reshape(P, -1)
        m = {
            "xb": xparts[b][0],
            "x8": xparts[b][1],
            "wq8T": f8(tiled_w(wqT)),
            "wkvT": bf(tiled_w(wkvT)),
            "woT": bf(woT),
        }
        in_maps.append(m)
    return in_maps, use_biases


def unshard(results, bo=None):
    """Sum head-group partials per batch (tensor-parallel unshard)."""
    out = np.zeros((BATCH, S, D), np.float32)
    for cid in range(8):
        b = cid // 4
        out[b] += np.asarray(results[cid]["out2"]).astype(np.float32)
    if bo is not None:
        bo = np.asarray(bo, np.float32)
        if np.any(bo):
            out += bo[None, None, :]
    return out


def kernel(x, wq, bq, wk, bk, wv, bv, wo, bo):
    in_maps, use_biases = make_in_maps(x, wq, bq, wk, bk, wv, bv, wo, bo)
    nc = _get_nc(use_biases)
    res = bass_utils.run_bass_kernel_spmd(nc, in_maps, core_ids=list(range(8)))
    return unshard(res.results, bo=bo)


# revision 23
# speedup vs baseline: 1.0903x; 1.0126x over previous
"""Trainium2 Bass kernel for ConvexLinearAttention (elu(x)+1 linear attention).

Full-input contract: kernel(**inputs) takes the unsharded tensors
(x [2,2048,1024], wq/wk/wv/wo [1024,1024], bq/bk/bv/bo [1024]) and returns the
full output [2,2048,1024].

Sharding (8 cores): data-parallel over batch (2) x head-group-parallel (4 groups
of 4 heads).  Each core projects only its 256-wide head slice, runs the
linearized attention (attended = qf @ (kf^T V) / (qf @ sum(kf)) -- an exact
refactoring of the dense normalized scores), and emits a partial output
projection in natural [s, e] layout.  The host sums the 4 head-group partials
per batch.

Precision plan (tolerance 2e-2, bf16 baseline err ~4e-3):
  - K|V projection stays bf16: V weight-quantization error is common-mode
    across sequence positions (x_bar @ dW) and does NOT average out in the
    attention mean -- fp8 V alone measured 3.3e-2.
  - Q projection: fp8e4 DoubleRow (wq8 stationary [128,2,128], x8 moving
    [128,2,512]).  Normalization cancels most q-side noise: 7.8e-3 alone.
  - Out projection: fp8e4 DoubleRow, one MM per (s-tile, e-half) pairing the
    two 128-wide head groups in the ko dim.  qs and M have disjoint dynamic
    ranges (qs ~ 7e-6, M ~ 1e2), so split scales: qs*2^17 (folded into the
    scaled ksum feeding the denominator matmul) and M*2^-6 (folded into the
    M PSUM evacuation); the out PSUM evacuation multiplies by 2^-11.
    Q+out fp8 together measured 1.7e-2 in simulation vs the 2e-2 gate.

DMA plan (per-queue throughput is descriptor-paced: ~35ns + ~60ns/KB per
descriptor, so both the dram AND sbuf side of every transfer must be
contiguous in 2KB+ runs):
  - xb (bf16 x for the K|V stationary) is tile-major on BOTH sides:
    dram rows (st p) hold [do s] 2KB runs, sbuf is [P, NST, ndt, P].
  - x8 (fp8 x, Q moving operand) is chunk-major: dram rows (sc p) hold
    [do s] 4KB runs, sbuf is [P, NSC, ndt, SC].
  - wkv streams as 2 halves on different queues so the dt-ordered matmul
    consumption starts after ~512KB.
  - Input triggers ride only sync/gpsimd: a dma_start occupies a slot in its
    engine's instruction stream and can block behind ring semaphores, so the
    scalar/vector queues (which carry the latency-critical feature ops) get
    no input DMAs after the first four.
  - 12 warm-up matmuls on resident data run during the input head so the PE
    HAM clock gate is already 8/8 when real work starts.
"""

from contextlib import ExitStack

import numpy as np
import ml_dtypes

import concourse.bass as bass
import concourse.mybir as mybir
import concourse.tile as tile
from concourse import bacc, bass_utils

F32 = mybir.dt.float32
BF16 = mybir.dt.bfloat16
FP8 = mybir.dt.float8e4
AF = mybir.ActivationFunctionType
ALU = mybir.AluOpType
DR = mybir.MatmulPerfMode.DoubleRow

NPBF = ml_dtypes.bfloat16
NPF8 = ml_dtypes.float8_e4m3  # IEEE-style e4m3 (max 240) = TRN float8e4

D = 1024          # model dim
S = 2048          # sequence length
BATCH = 2
CSL = 256         # head-slice width per core (4 heads x 64)
NG = 2            # 128-wide c-groups per core
P = 128
NST = S // P      # 16 s-tiles
SC = 512          # s-chunk
NSC = S // SC     # 4 s-chunks
STC = SC // P     # 4 s-tiles per chunk
EH = 512          # e-half width for out-proj rhs

Q_FP8 = True      # fp8 DoubleRow Q projection
# fp8 DoubleRow out projection (split scales).  Measured SLOWER than bf16:
# the single-MM-per-bank DoubleRow stream pays a serialized 256-col
# LDWEIGHTS per matmul (376ns vs 216) plus inter-MM gaps that let the HAM
# clock gate oscillate; the bf16 2-MM accumulation groups stream at 216ns
# with FWL-hidden weight loads.  Kept as a switch for reference.
OUT_FP8 = False
QS_SCALE = 2.0 ** 17   # qs -> qs * QS_SCALE (folded into scaled ksum)
M_SCALE = 2.0 ** -6    # M  -> M * M_SCALE  (folded into M PSUM evac)
O_SCALE = 1.0 / (QS_SCALE * M_SCALE)  # out evac multiplier (2^-11)
NWARM = 16

_CACHE: dict = {}


def install_ntff_hook_shim():
    """Provide ``antenv.axon_hooks`` when the image ships only the antenv stub.

    concourse.bass_utils imports it unconditionally on the axon trace path;
    without this shim trace=True (or BASS_TRACE=1) crashes.  Registers the real
    ctypes NTFF hook when the axon .so is present, else a None-returning stub
    so tracing degrades gracefully.
    """
    import os
    import sys
    import types

    if "antenv.axon_hooks" in sys.modules:
        return
    try:
        import antenv
        import antenv.axon_hooks  # noqa: F401
        return  # real module exists
    except ImportError:
        pass
    mod = types.ModuleType("antenv.axon_hooks")
    state: dict = {"h": None}
    mod.set_axon_ntff_profile_hook = lambda h: state.__setitem__("h", h)
    mod.get_axon_ntff_profile_hook = lambda: state.get("h")
    sys.modules["antenv.axon_hooks"] = mod
    antenv.axon_hooks = mod
    so_path = "/opt/axon/libaxon_pjrt.so"
    if os.path.exists(so_path):
        try:
            from trn_agent_boot.trn_boot import _ntff_profile_via_ctypes

            state["h"] = _ntff_profile_via_ctypes(so_path)
        except Exception:
            pass


def _build_kernel_body(ctx: ExitStack, tc: tile.TileContext, t, use_biases):
    nc = tc.nc
    # with biases, a 9th d-tile (ones row 0, zeros elsewhere) multiplies the
    # bias row appended to the weight matrices: exact bias add inside the GEMM
    ndt = 9 if use_biases else 8

    xbv = t["xb"].ap().rearrange("(st p) (do s) -> p st do s", p=P, do=8)
    x8v = t["x8"].ap().rearrange("(sc p) (do s) -> p sc do s", p=P, do=8)
    wq8T = t["wq8T"].ap().rearrange("p (do c) -> p do c", do=ndt)
    wkvT = t["wkvT"].ap().rearrange("p (do c) -> p do c", do=ndt)
    woT = t["woT"].ap().rearrange("p (g e) -> p g e", g=NG)
    out2 = t["out2"].ap().rearrange("(st p) e -> p st e", p=P)

    const = ctx.enter_context(tc.tile_pool(name="const", bufs=1))

    def single(shape, name, dtype=BF16):
        return const.tile(shape, dtype, name=name, tag=name)

    wkv_sb = single([P, ndt, 2 * CSL], "wkv_sb")
    wq8_sb = single([P, ndt, CSL], "wq8_sb", FP8)
    wo_sb = single([P, NG, D], "wo_sb")
    xb_sb = single([P, NST, ndt, P], "xb_sb")
    x8_sb = single([P, NSC, ndt, SC], "x8_sb", FP8)
    qf_sb = single([P, NG, S], "qf_sb")
    ones_sb = single([P, 1], "ones_sb")
    bkvT_sb = single([P, NG, P], "bkvT_sb")
    bden_sb = single([P, NG, P], "bden_sb")
    m_sb = single([P, NG, D], "m_sb", FP8 if OUT_FP8 else BF16)
    ksum_sb = single([P, NG], "ksum_sb")
    warm_sb = single([P, 2 * P], "warm_sb")
    # chunk-0 qs lives in the const pool: its den/recip/qs chain runs inside
    # phase A (between the two Q(3) groups) so the first out-proj matmul has
    # its stationary ready the moment the A-phase PSUM pools close
    qs0_sb = single([P, NG, SC], "qs0_sb", FP8 if OUT_FP8 else BF16)

    # ---- PE warm-up: a dozen matmuls on resident data keep the PE busy
    # through the HAM SHORT window during the input-DMA head, so the real
    # matmuls start at 2.4GHz instead of 1.2GHz.
    nc.vector.memset(warm_sb, 1.0)
    with tc.tile_pool(name="ps_warm", bufs=1, space="PSUM") as ps_w:
        warm_ps = ps_w.tile([P, 2 * P], F32, tag="warm_ps")
        for i in range(NWARM):
            nc.tensor.matmul(warm_ps, warm_sb[:, 0:P], warm_sb,
                             start=(i == 0), stop=(i == NWARM - 1))

    # ---- input DMA, need-ordered with per-queue parallelism.  The three
    # first-matmul dependencies lead the three DMA-capable queues so they
    # stream concurrently: wkv half 0 on sync (earliest to come up), xb
    # tile 0 on scalar, wkv half 1 on gpsimd.  Bulk trails in need order;
    # every descriptor is a 2-4KB contiguous run on both sides.
    h0 = (ndt + 1) // 2
    nc.sync.dma_start(out=wkv_sb[:, 0:h0, :], in_=wkvT[:, 0:h0, :])
    nc.scalar.dma_start(out=xb_sb[:, 0, 0:8, :], in_=xbv[:, 0, :, :])
    # gate the gpsimd bulk behind wkv half 0's arrival so the two critical
    # transfers get the fabric to themselves
    gate_sb = single([1, 1], "gate_sb")
    nc.gpsimd.tensor_copy(out=gate_sb, in_=wkv_sb[0:1, h0 - 1, 511:512])
    nc.gpsimd.dma_start(out=wkv_sb[:, h0:ndt, :], in_=wkvT[:, h0:ndt, :])
    nc.vector.memset(ones_sb, 1.0)
    if use_biases:
        nc.vector.memset(xb_sb[0:1, :, 8, :], 1.0)
        nc.vector.memset(xb_sb[1:P, :, 8, :], 0.0)
        nc.vector.memset(x8_sb[0:1, :, 8, :], 1.0)
        nc.vector.memset(x8_sb[1:P, :, 8, :], 0.0)
    nc.scalar.dma_start(out=xb_sb[:, 1, 0:8, :], in_=xbv[:, 1, :, :])
    nc.scalar.dma_start(out=wq8_sb, in_=wq8T)
    for sti in range(2, 4):
        nc.sync.dma_start(out=xb_sb[:, sti, 0:8, :], in_=xbv[:, sti, :, :])
    nc.gpsimd.dma_start(out=x8_sb[:, 0, 0:8, :], in_=x8v[:, 0, :, :])
    for sti in range(4, 8):
        nc.gpsimd.dma_start(out=xb_sb[:, sti, 0:8, :], in_=xbv[:, sti, :, :])
    nc.gpsimd.dma_start(out=x8_sb[:, 1, 0:8, :], in_=x8v[:, 1, :, :])
    for sti in range(8, 12):
        nc.gpsimd.dma_start(out=xb_sb[:, sti, 0:8, :], in_=xbv[:, sti, :, :])
    nc.gpsimd.dma_start(out=x8_sb[:, 2, 0:8, :], in_=x8v[:, 2, :, :])
    for sti in range(12, NST):
        nc.gpsimd.dma_start(out=xb_sb[:, sti, 0:8, :], in_=xbv[:, sti, :, :])
    nc.gpsimd.dma_start(out=x8_sb[:, 3, 0:8, :], in_=x8v[:, 3, :, :])
    nc.gpsimd.dma_start(out=wo_sb, in_=woT)
    nc.gpsimd.memset(bkvT_sb, 0.0)
    nc.gpsimd.memset(bden_sb, 0.0)

    # ---- phase A: K|V projection + feature map + KV^T/ksum accumulation,
    #      Q projection interleaved per s-chunk ----------------------------
    with tc.tile_pool(name="ps_kv", bufs=1, space="PSUM") as ps_kv:
        _phase_a(tc, nc, ps_kv, ndt, xb_sb, x8_sb, wkv_sb, wq8_sb, wo_sb,
                 qf_sb, ones_sb, bkvT_sb, bden_sb, m_sb, ksum_sb, qs0_sb)

    # ---- phase B: normalize q, out[s,e] = sum_g qs_g^T M_g ---------------
    # all den/recip/qs first (they only gate on ksum + qf), then the
    # out-projection streams PE-dense with copies pipelining behind it
    with tc.tile_pool(name="ps_d", bufs=1, space="PSUM") as ps_d, \
         tc.tile_pool(name="ps_o", bufs=7, space="PSUM") as ps_o, \
         tc.tile_pool(name="sb_qs", bufs=4) as sb_qs, \
         tc.tile_pool(name="sb_b", bufs=2) as sb_b:
        qs_tiles = [qs0_sb]
        for sc in range(1, NSC):
            csl = slice(sc * SC, (sc + 1) * SC)
            if OUT_FP8:
                qs = sb_qs.tile([P, NG, SC], FP8, tag="qs8")
            else:
                qs = sb_qs.tile([P, NG, SC], BF16, tag="qs16")
            for g in range(NG):
                d_ps = ps_d.tile([P, SC], F32, tag="d_ps")
                nc.tensor.matmul(d_ps, bden_sb[:, g, :], qf_sb[:, g, csl],
                                 start=True, stop=True)
                rden = sb_b.tile([P, SC], F32, tag=f"rden{g}")
                nc.vector.reciprocal_approx_fast(out=rden, in_=d_ps)
                # qs on gpsimd in 256-col halves: vector/scalar are fully
                # booked with the out-proj PSUM evacuations + reciprocals
                for qh in range(2):
                    hsl = slice(qh * (SC // 2), (qh + 1) * (SC // 2))
                    chsl = slice(sc * SC + qh * (SC // 2),
                                 sc * SC + (qh + 1) * (SC // 2))
                    nc.gpsimd.tensor_tensor(qs[:, g, hsl], qf_sb[:, g, chsl],
                                            rden[:, hsl], ALU.mult)
            qs_tiles.append(qs)
        for sc in range(NSC):
            _emit_outproj(nc, sb_b, ps_o, m_sb, out2, qs_tiles[sc], sc)


def _phase_a(tc, nc, ps_kv, ndt, xb_sb, x8_sb, wkv_sb, wq8_sb, wo_sb, qf_sb,
             ones_sb, bkvT_sb, bden_sb, m_sb, ksum_sb, qs0_sb):
    npair = ndt // 2
    odd = ndt % 2
    kvt_ps = [ps_kv.tile([P, P], F32, name=f"kvt_ps{g}", tag=f"kvt{g}")
              for g in range(NG)]
    # ksum columns for both groups share one bank => ONE accumulation group
    ksum_ps = ps_kv.tile([P, NG], F32, name="ksum_ps", tag="ksum")

    with tc.tile_pool(name="ps_a", bufs=3, space="PSUM") as ps_a, \
         tc.tile_pool(name="ps_q", bufs=2, space="PSUM") as ps_q, \
         tc.tile_pool(name="sb_a", bufs=3) as sb_a:

        def kv_accum(st, kf, v_sb):
            # KV^T / ksum accumulation per 128-group:
            #   kvt[cv, ck] += v[s, cv]^T kf[s, ck];  ksum[ck] += kf^T 1
            for g in range(NG):
                gsl = slice(g * P, (g + 1) * P)
                nc.tensor.matmul(
                    kvt_ps[g], v_sb[:, gsl], kf[:, gsl],
                    start=(st == 0), stop=(st == NST - 1))
                nc.tensor.matmul(
                    ksum_ps[:, g:g + 1], kf[:, gsl], ones_sb,
                    start=(st == 0 and g == 0),
                    stop=(st == NST - 1 and g == NG - 1))

        def kv_tile(st, defer_accum=False):
            # combined K|V projection: [s, 0:256]=K, [s, 256:512]=V
            kvp = ps_a.tile([P, 2 * CSL], F32, tag="kvp")
            for dt in range(ndt):
                nc.tensor.matmul(
                    kvp, xb_sb[:, st, dt, :], wkv_sb[:, dt, :],
                    start=(dt == 0), stop=(dt == ndt - 1))
            # kf = relu(K) + exp(min(K, 0))   (= elu(K)+1)
            kf = sb_a.tile([P, CSL], BF16, tag="kf")
            m_k = sb_a.tile([P, CSL], BF16, tag="m_k")
            nc.vector.tensor_scalar(m_k, kvp[:, 0:CSL], 0.0, None,
                                    op0=ALU.min)
            nc.scalar.activation(m_k, m_k, AF.Exp)
            nc.vector.scalar_tensor_tensor(
                kf, kvp[:, 0:CSL], 0.0, m_k, op0=ALU.max, op1=ALU.add)
            v_sb = sb_a.tile([P, CSL], BF16, tag="v_sb")
            nc.scalar.copy(out=v_sb, in_=kvp[:, CSL:2 * CSL])
            if defer_accum:
                return kf, v_sb
            kv_accum(st, kf, v_sb)
            return None

        def kv_chunk(sc):
            for sti in range(STC):
                kv_tile(sc * STC + sti)

        def q_group(sc, g):
            csl = slice(sc * SC, (sc + 1) * SC)
            if True:
                gsl = slice(g * P, (g + 1) * P)
                q_ps = ps_q.tile([P, SC], F32, tag="q_ps")
                if Q_FP8:
                    # DoubleRow: ko pairs of d-tiles, both operands fp8e4
                    for i in range(npair):
                        dsl = slice(2 * i, 2 * i + 2)
                        nc.tensor.matmul(
                            q_ps, wq8_sb[:, dsl, gsl], x8_sb[:, sc, dsl, :],
                            start=(i == 0), stop=(i == npair - 1 and not odd),
                            perf_mode=DR)
                    if odd:
                        nc.tensor.matmul(
                            q_ps, wq8_sb[:, ndt - 1, gsl],
                            x8_sb[:, sc, ndt - 1, :], start=False, stop=True)
                else:
                    for dt in range(ndt):
                        nc.tensor.matmul(
                            q_ps, wq8_sb[:, dt, gsl], x8_sb[:, sc, dt, :],
                            start=(dt == 0), stop=(dt == ndt - 1))
                m_q = sb_a.tile([P, SC], BF16, tag="m_q")
                nc.vector.tensor_scalar(m_q, q_ps, 0.0, None, op0=ALU.min)
                nc.scalar.activation(m_q, m_q, AF.Exp)
                nc.vector.scalar_tensor_tensor(
                    qf_sb[:, g, csl], q_ps, 0.0, m_q,
                    op0=ALU.max, op1=ALU.add)

        def q_chunk(sc):
            q_group(sc, 0)
            q_group(sc, 1)

        # A->B boundary choreography: any PE idle here lets the HAM clock
        # gate re-throttle and phase B then runs at 1.2GHz until it
        # re-warms.  The last tile's kvt/ksum wait on its feature-map evac
        # chain, so Q(3) g0 runs in between; the extraction copies execute
        # during Q(3) g1; chunk 0's den/recip/qs chain fills during the M
        # matmuls that follow.
        kv_chunk(0); q_chunk(0)
        kv_chunk(1); q_chunk(1)
        kv_chunk(2); q_chunk(2)
        for sti in range(STC - 1):
            kv_tile(3 * STC + sti)
        kf15, v15 = kv_tile(NST - 1, defer_accum=True)
        q_group(3, 0)
        kv_accum(NST - 1, kf15, v15)
        # bkvT[g][cv, ck] = KV^T for head(cv)==head(ck) else 0
        # bden[g][ck', ck] = ksum[ck']*s for head(ck')==head(ck) else 0
        # (s = 2^-17 when OUT_FP8 so rden = 2^17/den lands qs in fp8 range).
        # ksum -> SBUF once (vector reads PSUM), then the broadcast fills
        # run on gpsimd so the A-end vector queue (q3 feature ops) doesn't
        # delay the first den matmul.
        nc.vector.tensor_scalar(ksum_sb, ksum_ps,
                                1.0 / QS_SCALE if OUT_FP8 else 1.0, None,
                                op0=ALU.mult)
        for g in range(NG):
            for hb in range(2):
                hsl = slice(hb * 64, (hb + 1) * 64)
                nc.scalar.copy(out=bkvT_sb[hsl, g, hsl],
                               in_=kvt_ps[g][hsl, hsl])
                nc.gpsimd.tensor_copy(
                    out=bden_sb[hsl, g, hsl],
                    in_=ksum_sb[hsl, g:g + 1].to_broadcast((64, 64)))
        q_group(3, 1)
        # chunk-0 denominator chain, PE-covered by the M matmuls
        for g in range(NG):
            d_ps = ps_q.tile([P, SC], F32, tag="q_ps")
            nc.tensor.matmul(d_ps, bden_sb[:, g, :], qf_sb[:, g, 0:SC],
                             start=True, stop=True)
            rden0 = sb_a.tile([P, SC], F32, tag=f"rden0_{g}")
            nc.vector.reciprocal_approx_fast(out=rden0, in_=d_ps)
            for qh in range(2):
                hsl = slice(qh * (SC // 2), (qh + 1) * (SC // 2))
                nc.gpsimd.tensor_tensor(qs0_sb[:, g, hsl],
                                        qf_sb[:, g, hsl], rden0[:, hsl],
                                        ALU.mult)

    with tc.tile_pool(name="ps_m", bufs=2, space="PSUM") as ps_m:
        for g in range(NG):
            for eh in range(2):
                esl = slice(eh * EH, (eh + 1) * EH)
                m_ps = ps_m.tile([P, EH], F32, tag="m_ps")
                nc.tensor.matmul(m_ps, bkvT_sb[:, g, :], wo_sb[:, g, esl],
                                 start=True, stop=True)
                scale = M_SCALE if OUT_FP8 else 1.0
                if eh == 0:
                    nc.scalar.activation(m_sb[:, g, esl], m_ps, AF.Copy,
                                         scale=scale)
                else:
                    nc.vector.tensor_scalar(m_sb[:, g, esl], m_ps, scale,
                                            None, op0=ALU.mult)


def _emit_outproj(nc, sb_b, ps_o, m_sb, out2, qs, sc):
    # per-s-tile 256KB output DMAs alternating sync/gpsimd: each departs
    # after only its own two PSUM evacuations (subtile deps), keeping the
    # output stream dense through phase B and the post-compute drain short
    for half in range(2):
        o_sb = sb_b.tile([P, STC // 2, D], BF16, tag=f"o_sb{half}")
        for hi in range(STC // 2):
            sti = half * 2 + hi
            tsl = slice(sti * P, (sti + 1) * P)
            for eh in range(2):
                esl = slice(eh * EH, (eh + 1) * EH)
                o_ps = ps_o.tile([P, EH], F32, tag="o_ps")
                if OUT_FP8:
                    # one DoubleRow MM sums both head groups via the ko dim
                    nc.tensor.matmul(o_ps, qs[:, :, tsl], m_sb[:, :, esl],
                                     start=True, stop=True, perf_mode=DR)
                else:
                    for g in range(NG):
                        nc.tensor.matmul(
                            o_ps, qs[:, g, tsl], m_sb[:, g, esl],
                            start=(g == 0), stop=(g == NG - 1))
                # PSUM evacuation balanced 4/4 over scalar/vector per chunk
                oscale = O_SCALE if OUT_FP8 else 1.0
                if eh == 0:
                    nc.scalar.activation(o_sb[:, hi, esl], o_ps, AF.Copy,
                                         scale=oscale)
                else:
                    nc.vector.tensor_scalar(o_sb[:, hi, esl], o_ps, oscale,
                                            None, op0=ALU.mult)
            # outputs ride sync/scalar: gpsimd's queue carries the qs
            # multiplies, and a DMA trigger's ring-semaphore wait would
            # stall them (and with them the out-proj matmuls)
            gsti = sc * STC + sti
            eng = nc.sync if gsti % 2 == 0 else nc.scalar
            eng.dma_start(out=out2[:, gsti:gsti + 1, :],
                          in_=o_sb[:, hi:hi + 1, :])


def build_nc(use_biases):
    nc = bacc.Bacc("TRN2", target_bir_lowering=False, debug=False)
    ndt = 9 if use_biases else 8
    t = {}
    t["xb"] = nc.dram_tensor("xb", [NST * P, 8 * P], BF16,
                             kind="ExternalInput")
    t["x8"] = nc.dram_tensor("x8", [NSC * P, 8 * SC], FP8,
                             kind="ExternalInput")
    t["wq8T"] = nc.dram_tensor("wq8T", [P, ndt * CSL], FP8,
                               kind="ExternalInput")
    t["wkvT"] = nc.dram_tensor("wkvT", [P, ndt * 2 * CSL], BF16,
                               kind="ExternalInput")
    t["woT"] = nc.dram_tensor("woT", [P, NG * D], BF16, kind="ExternalInput")
    t["out2"] = nc.dram_tensor("out2", [S, D], BF16, kind="ExternalOutput")

    with tile.TileContext(nc) as tc:
        with ExitStack() as ctx:
            _build_kernel_body(ctx, tc, t, use_biases)
    nc.compile()
    return nc


def _get_nc(use_biases):
    key = ("nc", use_biases)
    if key not in _CACHE:
        _CACHE[key] = build_nc(use_biases)
    return _CACHE[key]


def make_in_maps(x, wq, bq, wk, bk, wv, bv, wo, bo, use_biases=None):
    """Shard the full inputs into the 8 per-core input maps."""
    f = lambda a: np.asarray(a, dtype=np.float32)
    x, wq, bq, wk, bk = f(x), f(wq), f(bq), f(wk), f(bk)
    wv, bv, wo, bo = f(wv), f(bv), f(wo), f(bo)
    if use_biases is None:
        # bo is applied host-side in unshard(); bq/bk/bv need the in-GEMM path
        use_biases = any(np.any(b) for b in (bq, bk, bv))
    bf = lambda a: np.ascontiguousarray(a).astype(NPBF)
    f8 = lambda a: np.ascontiguousarray(a).astype(NPF8)
    ndt = 9 if use_biases else 8

    def tiled_w(wT):
        # [ndt*128, c] -> [128, ndt*c]: per-partition contiguous weight rows
        c = wT.shape[1]
        return wT.reshape(ndt, P, c).transpose(1, 0, 2).reshape(P, ndt * c)

    xparts = {}
    for b in range(BATCH):
        xt = np.ascontiguousarray(x[b].T)            # [D, S]
        v4 = xt.reshape(8, P, NST, P)
        # tile-major: row (st*P + p) = [do, s] -- 2KB contiguous per row
        xb_t = v4.transpose(2, 1, 0, 3).reshape(NST * P, 8 * P)
        v5 = xt.reshape(8, P, NSC, SC)
        # chunk-major: row (sc*P + p) = [do, s] -- 4KB contiguous per row
        x8_t = v5.transpose(2, 1, 0, 3).reshape(NSC * P, 8 * SC)
        xparts[b] = (bf(xb_t), f8(x8_t))

    in_maps = []
    for cid in range(8):
        b, hg = divmod(cid, 4)
        hs = slice(hg * CSL, (hg + 1) * CSL)
        wkvT = np.concatenate([wk[hs, :].T, wv[hs, :].T], axis=1)
        wqT = wq[hs, :].T
        if use_biases:
            # bias row at row D (multiplied by the on-chip ones row), zero
            # padding to the 9*128 augmented contraction size
            wkvT = np.concatenate(
                [wkvT, np.concatenate([bk[hs], bv[hs]])[None, :],
                 np.zeros((P - 1, 2 * CSL), np.float32)], axis=0)
            wqT = np.concatenate(
                [wqT, bq[hs][None, :], np.zeros((P - 1, CSL), np.float32)],
                axis=0)
        woT = wo[:, hs].T.reshape(NG, P, D).transpose(1, 0, 2).reshape(P, -1)
        m = {
            "xb": xparts[b][0],
            "x8": xparts[b][1],
            "wq8T": f8(tiled_w(wqT)),
            "wkvT": bf(tiled_w(wkvT)),
            "woT": bf(woT),
        }
        in_maps.append(m)
    return in_maps, use_biases


def unshard(results, bo=None):
    """Sum head-group partials per batch (tensor-parallel unshard)."""
    out = np.zeros((BATCH, S, D), np.float32)
    for cid in range(8):
        b = cid // 4
        out[b] += np.asarray(results[cid]["out2"]).astype(np.float32)
    if bo is not None:
        bo = np.asarray(bo, np.float32)
        if np.any(bo):
            out += bo[None, None, :]
    return out


def kernel(x, wq, bq, wk, bk, wv, bv, wo, bo):
    in_maps, use_biases = make_in_maps(x, wq, bq, wk, bk, wv, bv, wo, bo)
    nc = _get_nc(use_biases)
    res = bass_utils.run_bass_kernel_spmd(nc, in_maps, core_ids=list(range(8)))
    return unshard(res.results, bo=bo)
